# revision 56
# baseline (speedup 1.0000x reference)
"""BiLSTM-CRF loss kernel for trn2, one core = 32 sequences (data parallel).

Algorithm (validated in proto.py / proto_bf16.py):
- embedding gather via dma_gather(transpose) -> x (128=E, ntok) bf16, t-major
- BiLSTM: all-sigmoid gates (tanh(x)=2*sigma(2x)-1 folded into g-gate weights),
  gates psum accumulate: xproj window matmuls + rank-1 bias + per-step Whh mm
- emissions chunked: psum = WoutT halves @ H, EM = exp(psum + bout - log T)
- numerator: A' = sum OHM*(emis + end x laststep)  [TTR per chunk],
  B+C via (49,48) pair histogram (one-hot matmuls) . trans_ext
- CRF denominator in exp space: alpha chain (t=0..L/2-1) and backward G chain
  (t=L-1..L/2-1) with EEND x laststep injection; denom = sum log(dot) +
  log(T) * masksum
Output per core: (1, 8) f32: [0]=numerator partial sum, [1]=denominator partial.
loss = (sum_den - sum_num) / B   (host combines the 8 cores)
"""
import numpy as np
import ml_dtypes

import concourse.bacc as bacc
import concourse.mybir as mybir
from concourse.tile import TileContext

BF16 = ml_dtypes.bfloat16
F32 = np.float32
AF = mybir.ActivationFunctionType
ALU = mybir.AluOpType
DT = mybir.dt

T = 48

# degree-5 odd minimax fit of tanh on [-1.05, 1.05] (|c| stays below ~0.95;
# max poly err 5.1e-4, under the bf16 rounding already present on H)
TANH_A0 = 0.9964321688911302
TANH_A1 = -0.30413271171334516
TANH_A2 = 0.06905329873698512


def _register_dve_ops():
    """Self-register two fused DVE ops (per-NEFF table, no firmware change):
    TANH_MUL_ANT: out = tanhpoly(in0) * in1   (kills TANH ACT + mult TT)
    AFFINE_MUL_ANT: out = (in0*s0 + s1) * in1 (affine_mul_reduce minus the
    accum drain op)."""
    import dataclasses
    import concourse.dve_ops as D
    from concourse.dve_spec import Spec, Src0, Src1, C0, C1, C2, sq, lower
    from concourse.dve_ops import has_src1
    from concourse.dve_uop import DveOpSpec

    def _register(name, spec, subdim=False):
        for o in D.OPS:
            if o.name == name:
                return o
        op = D.DveOp(name, spec, subdim=subdim, uops_sha={})
        D.OPS.append(op)
        D.CUSTOM_DVE_SPECS[name] = spec
        D._SUB_OPCODE_FOR_NAME[name] = D._CUSTOM_DVE_ROW_BASE + len(D.OPS) - 1
        shas = {}
        for ver in ("v3", "v4"):
            s = DveOpSpec(name=name, opcode=D.get_dve_sub_opcode(name),
                          uops=lower(spec, ver=ver), rd1_en=has_src1(spec))
            shas[ver] = s.sha(ver)
        op2 = dataclasses.replace(op, uops_sha=shas)
        D.OPS[-1] = op2
        D.CUSTOM_DVE_SPECS[name] = op2.spec
        return op2

    u = sq(Src0)
    tanh_mul = _register(
        "TANH_MUL_ANT",
        Spec(
            body=Src0 * (C0 + u * (C1 + u * C2)) * Src1,
            reference=lambda in0, in1, s0, s1, imm2: (
                in0.astype(np.float32)
                * (s0 + in0.astype(np.float32) ** 2
                   * (s1 + in0.astype(np.float32) ** 2 * imm2))
                * in1.astype(np.float32)),
        ))
    aff_mul = _register(
        "AFFINE_MUL_ANT",
        Spec(
            body=(Src0 * C0 + C1) * Src1,
            reference=lambda in0, in1, s0, s1, imm2: (
                (in0.astype(np.float32) * s0 + s1) * in1.astype(np.float32)),
        ))
    return tanh_mul, aff_mul


# --------------------------------------------------------------------------
# host-side preparation
# --------------------------------------------------------------------------

def prep_params(inp):
    """Build replicated parameter arrays (numpy) from raw inputs."""
    p = {}
    p["emb"] = np.ascontiguousarray(inp["emb"]).astype(BF16)

    def mk(Wih, Whh, bih, bhh):
        def reorder(W):
            i, f, g, o = np.split(np.asarray(W, F32), 4, 0)
            return np.concatenate([i, f, o, 2.0 * g], 0)
        WihT = np.ascontiguousarray(reorder(Wih).T).astype(BF16)   # (128, 512)
        WhhT = np.ascontiguousarray(reorder(Whh).T).astype(BF16)   # (128, 512)
        b = np.asarray(bih, F32) + np.asarray(bhh, F32)
        bi, bf_, bg, bo = np.split(b, 4)
        bias = np.concatenate([bi, bf_, bo, 2.0 * bg]).reshape(1, -1).astype(BF16)
        return WihT, WhhT, bias

    p["wiht_f"], p["whht_f"], p["bias_f"] = mk(inp["Wih_f"], inp["Whh_f"], inp["bih_f"], inp["bhh_f"])
    p["wiht_b"], p["whht_b"], p["bias_b"] = mk(inp["Wih_b"], inp["Whh_b"], inp["bih_b"], inp["bhh_b"])
    Wout = np.asarray(inp["Wout"], F32)     # (48, 256)
    H = Wout.shape[1] // 2
    p["wot_f"] = np.ascontiguousarray(Wout[:, :H].T).astype(BF16)   # (128, 48)
    p["wot_b"] = np.ascontiguousarray(Wout[:, H:].T).astype(BF16)
    c0 = np.log(T)
    p["exbias"] = (np.asarray(inp["bout"], F32) - c0).reshape(T, 1).astype(F32)
    trans = np.asarray(inp["trans"], F32)
    p["et"] = np.exp(trans).astype(BF16)                     # (48,48) lhsT alpha
    p["ett"] = np.ascontiguousarray(np.exp(trans).T).astype(BF16)  # lhsT G
    p["estart"] = np.exp(np.asarray(inp["start_trans"], F32)).reshape(T, 1).astype(F32)
    p["eendrow"] = np.exp(np.asarray(inp["end_trans"], F32)).reshape(1, T).astype(BF16)
    # bout is absent from the emission psum that the A-gather reads (it only
    # enters via the exp bias), so fold it in via the pair-histogram: every
    # masked position contributes exactly one CNT count with its cur-tag.
    # Row 50 counts last-step positions (ls indicator in ohp col 50), which
    # carries end_trans into the numerator without per-chunk endrow matmuls.
    p["transext"] = np.concatenate([
        trans + np.asarray(inp["bout"], F32)[None, :],
        np.asarray(inp["start_trans"], F32)[None, :]
        + np.asarray(inp["bout"], F32)[None, :],
        np.zeros((1, T), F32),
        np.asarray(inp["end_trans"], F32)[None, :],
    ], 0).astype(F32)  # (51, 48): trans+bout, start+bout, zero, end
    # gate-region indicator for the rank-4 bias matmul: (4, 1024) over the
    # psum window [4 gates x REG=256 f32 cols]; ind[g, c] = (c // 256 == g)
    reg = 256
    p["ind"] = (np.arange(4 * reg)[None, :] // reg ==
                np.arange(4)[:, None]).astype(BF16)
    return p


def prep_shard(words, tags, mask):
    """Per-core input arrays. words/tags/mask: (b, L)."""
    b, L = words.shape
    ntok = b * L
    npch = ntok // 128
    w_tm = np.ascontiguousarray(words.T).reshape(-1)
    tags_tm = np.ascontiguousarray(tags.T).reshape(-1)
    m_tm = np.ascontiguousarray(mask.T).reshape(-1).astype(F32)

    d = {}
    gi = w_tm.astype(np.int16).reshape(ntok // 16, 16).T          # (16, ntok/16)
    d["gidx"] = np.ascontiguousarray(np.tile(gi, (8, 1))).astype(np.int16)
    tm_masked = np.where(m_tm > 0, tags_tm, 99).astype(F32)
    tprev = np.concatenate([np.full(b, 48, F32), tags_tm[:-b].astype(F32)])
    d["mask_pc"] = np.ascontiguousarray(m_tm.reshape(-1, 128).T).astype(BF16)
    m_pad = np.pad(m_tm, (0, b))
    ls = (m_tm - m_pad[b:]).astype(F32)
    d["lsrow"] = ls.astype(BF16).reshape(1, ntok)
    # host-prepped one-hots (pure functions of tags/mask):
    # ohm: (48, ntok) onehot of masked cur-tag, token-chunk layout (A-gather)
    d["ohm"] = (np.arange(T, dtype=F32)[:, None] ==
                tm_masked[None, :]).astype(BF16)
    # ohp: (128, 64*npch) pcol onehot of prev-tag (48 = seq start) with the
    # last-step indicator in col 50; ohc: (128, 48*npch) masked cur-tag
    ohp = (tprev[:, None] == np.arange(64, dtype=F32)[None, :]).astype(F32)
    ohp[:, 50] = ls
    d["ohp"] = np.ascontiguousarray(
        ohp.reshape(npch, 128, 64).transpose(1, 0, 2).reshape(128, npch * 64)
    ).astype(BF16)
    ohc = (tm_masked[:, None] == np.arange(T, dtype=F32)[None, :]).astype(F32)
    d["ohc"] = np.ascontiguousarray(
        ohc.reshape(npch, 128, T).transpose(1, 0, 2).reshape(128, npch * T)
    ).astype(BF16)
    return d


# --------------------------------------------------------------------------
# device kernel builder
# --------------------------------------------------------------------------

def build(L=512, BLOC=32, W=8, V=32000, debug=False, phases=("lstm", "hist", "emis", "crf")):
    ntok = L * BLOC
    NW = L // W
    half = L // 2
    NCH = ntok // 512          # emission chunks
    NPCH = ntok // 128         # one-hot pchunks
    c0 = float(np.log(T))

    tanh_mul, aff_mul = _register_dve_ops()
    nc = bacc.Bacc()
    dp = nc.declare_dram_parameter
    g_gidx = dp("gidx", [128, ntok // 16], DT.int16, isOutput=False)
    g_ohm = dp("ohm", [T, ntok], DT.bfloat16, isOutput=False)
    g_ohp = dp("ohp", [128, 64 * NPCH], DT.bfloat16, isOutput=False)
    g_ohc = dp("ohc", [128, T * NPCH], DT.bfloat16, isOutput=False)
    g_maskpc = dp("mask_pc", [128, ntok // 128], DT.bfloat16, isOutput=False)
    g_lsrow = dp("lsrow", [1, ntok], DT.bfloat16, isOutput=False)
    g_emb = dp("emb", [V, 128], DT.bfloat16, isOutput=False)
    g_w = {}
    for nm in ("wiht_f", "whht_f", "wiht_b", "whht_b"):
        g_w[nm] = dp(nm, [128, 512], DT.bfloat16, isOutput=False)
    g_bias = {d: dp(f"bias_{d}", [1, 512], DT.bfloat16, isOutput=False) for d in "fb"}
    g_wot = {d: dp(f"wot_{d}", [128, T], DT.bfloat16, isOutput=False) for d in "fb"}
    g_exbias = dp("exbias", [T, 1], DT.float32, isOutput=False)
    g_et = dp("et", [T, T], DT.bfloat16, isOutput=False)
    g_ett = dp("ett", [T, T], DT.bfloat16, isOutput=False)
    g_estart = dp("estart", [T, 1], DT.float32, isOutput=False)
    g_eendrow = dp("eendrow", [1, T], DT.bfloat16, isOutput=False)
    g_transext = dp("transext", [51, T], DT.float32, isOutput=False)
    g_out = dp("out", [1, 8], DT.float32, isOutput=True)
    if debug:
        g_dbg1 = dp("dbg1", [T, 512], DT.float32, isOutput=True)
        g_dbg2 = dp("dbg2", [T, 512], DT.float32, isOutput=True)
        g_dbg3 = dp("dbg3", [T, 16], DT.float32, isOutput=True)

    with TileContext(nc) as tc:
        with tc.tile_pool(name="persist", bufs=1) as pp:
            # ---- persistent SBUF tiles
            Hf = pp.tile([128, ntok], DT.bfloat16, tag="Hf", name="Hf")
            Hb = pp.tile([128, ntok], DT.bfloat16, tag="Hb", name="Hb")
            wiht = {}
            whht = {}
            bias = {}
            wot = {}
            for d in "fb":
                wiht[d] = pp.tile([128, 512], DT.bfloat16, tag=f"wiht{d}", name=f"wiht{d}")
                whht[d] = pp.tile([128, 512], DT.bfloat16, tag=f"whht{d}", name=f"whht{d}")
                bias[d] = pp.tile([1, 512], DT.bfloat16, tag=f"bias{d}", name=f"bias{d}")
                wot[d] = pp.tile([128, T], DT.bfloat16, tag=f"wot{d}", name=f"wot{d}")
            exbias = pp.tile([T, 1], DT.float32, tag="exbias", name="exbias")
            et_sb = pp.tile([T, T], DT.bfloat16, tag="et", name="et")
            ett_sb = pp.tile([T, T], DT.bfloat16, tag="ett", name="ett")
            estart = pp.tile([T, 1], DT.float32, tag="estart", name="estart")
            eendrow = pp.tile([1, T], DT.bfloat16, tag="eendrow", name="eendrow")
            transext = pp.tile([51, T], DT.float32, tag="transext", name="transext")
            ohm_sb = pp.tile([T, ntok], DT.bfloat16, tag="ohm", name="ohm")
            ohp_sb = pp.tile([128, 64 * NPCH], DT.bfloat16, tag="ohp", name="ohp")
            ohc_sb = pp.tile([128, T * NPCH], DT.bfloat16, tag="ohc", name="ohc")
            m_pcol = pp.tile([128, NPCH], DT.bfloat16, tag="mpcol", name="mpcol")
            lsrow = pp.tile([1, ntok], DT.bfloat16, tag="lsrow", name="lsrow")
            # small constants
            ones48row = pp.tile([1, T], DT.float32, tag="ones48row", name="ones48row")
            ones128row = pp.tile([1, 128], DT.float32, tag="ones128row", name="ones128row")
            onesrow512 = pp.tile([1, 512], DT.float32, tag="onesrow512", name="onesrow512")
            onesrow512b = pp.tile([1, 512], DT.bfloat16, tag="onesrow512b", name="onesrow512b")
            ones48col = pp.tile([T, 1], DT.float32, tag="ones48col", name="ones48col")
            ones51col = pp.tile([51, 1], DT.float32, tag="ones51col", name="ones51col")
            ones128col = pp.tile([128, 1], DT.float32, tag="ones128col", name="ones128col")
            # LSTM state
            cst = {d: pp.tile([128, BLOC], DT.float32, tag=f"c{d}", name=f"c{d}") for d in "fb"}
            tmp1 = {d: pp.tile([128, BLOC], DT.float32, tag=f"tmp1{d}", name=f"tmp1{d}") for d in "fb"}
            tmp2 = {d: pp.tile([128, BLOC], DT.bfloat16, tag=f"tmp2{d}", name=f"tmp2{d}") for d in "fb"}
            tct = {d: pp.tile([128, BLOC], DT.bfloat16, tag=f"tct{d}", name=f"tct{d}") for d in "fb"}
            jacc = {d: pp.tile([128, 1], DT.float32, tag=f"jacc{d}", name=f"jacc{d}") for d in "fb"}
            # numerator accumulators
            accA = pp.tile([T, NCH], DT.float32, tag="accA", name="accA")
            accA_red = pp.tile([T, 1], DT.float32, tag="accAred", name="accAred")
            accBC = pp.tile([51, 1], DT.float32, tag="accBC", name="accBC")
            junkA = pp.tile([T, 512], DT.bfloat16, tag="junkA", name="junkA")
            junkBC = pp.tile([51, T], DT.float32, tag="junkBC", name="junkBC")
            msum = pp.tile([128, 1], DT.float32, tag="msum", name="msum")
            # CRF tiles
            ea = [pp.tile([T, BLOC], DT.bfloat16, tag=f"ea{i}", name=f"ea{i}") for i in range(2)]
            emg = pp.tile([T, BLOC], DT.bfloat16, tag="emg", name="emg")
            dott = pp.tile([T, BLOC], DT.float32, tag="dott", name="dott")
            logrow = pp.tile([1, BLOC], DT.float32, tag="logrow", name="logrow")
            dsum = pp.tile([1, 1], DT.float32, tag="dsum", name="dsum")
            tmp11 = pp.tile([1, 1], DT.float32, tag="tmp11", name="tmp11")
            out_sb = pp.tile([1, 8], DT.float32, tag="outsb", name="outsb")

            # ---- input DMAs
            S = nc.sync
            for d in "fb":
                S.dma_start(out=wiht[d][:], in_=g_w[f"wiht_{d}"][:])
                S.dma_start(out=whht[d][:], in_=g_w[f"whht_{d}"][:])
                S.dma_start(out=bias[d][:], in_=g_bias[d][:])
                S.dma_start(out=wot[d][:], in_=g_wot[d][:])
            S.dma_start(out=exbias[:], in_=g_exbias[:])
            S.dma_start(out=et_sb[:], in_=g_et[:])
            S.dma_start(out=ett_sb[:], in_=g_ett[:])
            S.dma_start(out=estart[:], in_=g_estart[:])
            S.dma_start(out=eendrow[:], in_=g_eendrow[:])
            S.dma_start(out=transext[:], in_=g_transext[:])
            S.dma_start(out=m_pcol[:], in_=g_maskpc[:])
            S.dma_start(out=lsrow[:], in_=g_lsrow[:])

            # constants
            Vv = nc.vector
            Sc = nc.scalar
            Vv.memset(ones48row[:], 1.0)
            Vv.memset(ones128row[:], 1.0)
            Vv.memset(onesrow512[:], 1.0)
            Vv.memset(onesrow512b[:], 1.0)
            Vv.memset(ones48col[:], 1.0)
            Vv.memset(ones51col[:], 1.0)
            Vv.memset(ones128col[:], 1.0)
            Vv.memset(accA[:], 0.0)
            Vv.memset(out_sb[:], 0.0)
            for d in "fb":
                Vv.memset(cst[d][:], 0.0)

            # ---------------- LSTM ----------------
            emis_lvl = 4
            for ph in phases:
                if ph.startswith("emis") and len(ph) > 4:
                    emis_lvl = int(ph[4:])
            do_lstm = "lstm" in phases
            do_hist = "hist" in phases
            do_emis = "emis" in phases
            do_crf = "crf" in phases
            do_emis = do_emis or any(ph.startswith("emis") for ph in phases)
            if not do_lstm:
                Vv.memset(Hf[:], 0.0)
                Vv.memset(Hb[:], 0.0)
            REG = 32 * W      # region width per gate
            Hdir = {"f": Hf, "b": Hb}
            # issue the embedding gathers first (gpsimd queue), then the bulky
            # one-hot DMAs so they don't delay the gather start
            xpool_cm = tc.tile_pool(name="xpool", bufs=1)
            xp = xpool_cm.__enter__()
            x = xp.tile([128, ntok], DT.bfloat16, tag="x", name="x")
            gidx = xp.tile([128, ntok // 16], DT.int16, tag="gidx", name="gidx")
            S.dma_start(out=gidx[:], in_=g_gidx[:])
            GCH = min(ntok, 1024)
            _ng = ntok // GCH
            _order = []
            for _i in range((_ng + 1) // 2):
                _order.append(_i)
                if _ng - 1 - _i != _i:
                    _order.append(_ng - 1 - _i)
            for gc in _order:
                nc.gpsimd.dma_gather(
                    out_ap=x[:, gc * GCH:(gc + 1) * GCH].rearrange(
                        "p (o n) -> p o n", o=1),
                    in_ap=g_emb[:],
                    idxs_ap=gidx[:, gc * (GCH // 16):(gc + 1) * (GCH // 16)],
                    num_idxs=GCH,
                    num_idxs_reg=GCH,
                    elem_size=128,
                    transpose=True,
                    single_packet=False,
                )
            S.dma_start(out=ohp_sb[:], in_=g_ohp[:])
            S.dma_start(out=ohc_sb[:], in_=g_ohc[:])
            S.dma_start(out=ohm_sb[:], in_=g_ohm[:])
            # pair histogram during the gather window (tensor is mostly idle
            # here; the psum bank is freed before the LSTM pool opens)
            with tc.tile_pool(name="cnt_ps", bufs=1, space="PSUM") as cpsp:
                cntps = cpsp.tile([64, T], DT.float32, name="cntps")
                for q in range(NPCH):
                    nc.tensor.matmul(cntps[:], ohp_sb[:, 64 * q:64 * (q + 1)],
                                     ohc_sb[:, T * q:T * (q + 1)],
                                     start=(q == 0), stop=(q == NPCH - 1),
                                     skip_group_check=True)
                Vv.affine_mul_reduce(
                    out=junkBC[:], accum_out=accBC[:],
                    in0=transext[:], in1=cntps[0:51, :],
                    scale=1.0, bias=0.0)
            with tc.tile_pool(name="lstm_ps", bufs=2, space="PSUM") as lpsp, \
                 tc.tile_pool(name="lstm_sb", bufs=3) as lsb:
                def alloc_boundary(w):
                    """Allocate window-w gate psums; return per-gate groups of
                    deferred xproj/bias matmul closures. Each group is gated on
                    a junk 1-col copy from the current H column (WAW hazard on
                    the gate region, erased by the group's start=True matmul)
                    so the greedy scheduler cannot run them all at window
                    start and starve the recurrence of tensor slots."""
                    pfd = {}
                    x0d = {}
                    for d in "fb":
                        pfd[d] = lpsp.tile([128, 4 * REG], DT.float32,
                                           tag=f"pf{d}", name=f"pf{d}")
                        x0d[d] = (w * W * BLOC) if d == "f" \
                            else (L - (w + 1) * W) * BLOC
                    groups = []
                    for gi in range(4):
                        for d in "fb":
                            cls = []

                            def xmm(d=d, gi=gi, pfd=pfd, x0d=x0d):
                                nc.tensor.matmul(
                                    pfd[d][:, gi * REG:(gi + 1) * REG],
                                    wiht[d][:, gi * 128:(gi + 1) * 128],
                                    x[:, x0d[d]:x0d[d] + W * BLOC],
                                    start=((gi * REG * 4) % 2048 == 0),
                                    stop=False, skip_group_check=True)

                            def bmm(d=d, gi=gi, pfd=pfd):
                                nc.tensor.matmul(
                                    pfd[d][:, gi * REG:(gi + 1) * REG],
                                    bias[d][0:1, gi * 128:(gi + 1) * 128],
                                    onesrow512b[0:1, 0:W * BLOC],
                                    start=False, stop=False, skip_group_check=True)
                            cls.append(xmm)
                            cls.append(bmm)
                            groups.append((d, gi, cls))
                    return pfd, groups

                pf = None
                if do_lstm:
                    pf, groups0 = alloc_boundary(0)
                    for _, _, cls in groups0:
                        for c in cls:
                            c()
                for w in range(NW if do_lstm else 0):
                    if w + 1 < NW:
                        pf_next, pending = alloc_boundary(w + 1)
                    else:
                        pf_next, pending = None, []
                    for s in range(W):
                        for d in "fb":
                            if d == "f":
                                t = w * W + s
                                slot = s
                                tprev_col = (t - 1) * BLOC
                                first = (t == 0)
                            else:
                                t = L - 1 - (w * W + s)
                                slot = W - 1 - s
                                tprev_col = (t + 1) * BLOC
                                first = (t == L - 1)
                            Hd = Hdir[d]
                            pfd = pf[d]
                            if not first:
                                for gi in range(4):
                                    nc.tensor.matmul(
                                        pfd[:, gi * REG + slot * 32: gi * REG + (slot + 1) * 32],
                                        whht[d][:, gi * 128:(gi + 1) * 128],
                                        Hd[:, tprev_col:tprev_col + BLOC],
                                        start=False, stop=True, skip_group_check=True)
                            # sigma over the 4 gate slices
                            Sg = lsb.tile([128, 128], DT.bfloat16, tag=f"S{d}", name=f"S{d}")
                            pf3 = pfd[:].rearrange("p (g n) -> p g n", g=4)
                            Sc.activation(
                                Sg[:].rearrange("p (g n) -> p g n", g=4),
                                pf3[:, :, slot * 32:(slot + 1) * 32],
                                AF.Sigmoid)
                            # c update
                            if first:
                                Vv._custom_dve(
                                    aff_mul, out=cst[d][:],
                                    in0=Sg[:, 96:128], in1=Sg[:, 0:32],
                                    s0=2.0, s1=-1.0)
                            else:
                                Vv.tensor_tensor(out=tmp1[d][:], in0=Sg[:, 32:64],
                                                 in1=cst[d][:], op=ALU.mult)
                                Vv._custom_dve(
                                    aff_mul, out=tmp2[d][:],
                                    in0=Sg[:, 96:128], in1=Sg[:, 0:32],
                                    s0=2.0, s1=-1.0)
                                Vv.tensor_tensor(out=cst[d][:], in0=tmp1[d][:],
                                                 in1=tmp2[d][:], op=ALU.add)
                            # h = sigma(o) * tanh(c) in one fused DVE op
                            Vv._custom_dve(
                                tanh_mul, out=Hd[:, t * BLOC:(t + 1) * BLOC],
                                in0=cst[d][:], in1=Sg[:, 64:96],
                                s0=TANH_A0, s1=TANH_A1, imm2=TANH_A2)
                        # release one per-dir gate-group of next-window
                        # xproj/bias matmuls, gated on this step's fresh H col
                        # (junk copy on the scalar engine, which has slack)
                        if pending:
                            gd, gi, cls = pending.pop(0)
                            tcur = (w * W + s) if gd == "f" \
                                else (L - 1 - (w * W + s))
                            Sc.activation(
                                pf_next[gd][:, gi * REG:gi * REG + 1],
                                Hdir[gd][:, tcur * BLOC:tcur * BLOC + 1],
                                AF.Copy)
                            for c in cls:
                                c()
                    pf = pf_next

            xpool_cm.__exit__(None, None, None)   # x is dead after the LSTM
            empool = tc.tile_pool(name="empool", bufs=1)
            emp = empool.__enter__()
            EM = emp.tile([T, ntok], DT.bfloat16, tag="EM", name="EM")
            # ------- merged emissions + CRF (interleaved) -------
            # EM chunks are emitted from both ends inward so the alpha chain
            # (consuming t ascending) and the G chain (descending) can start
            # right after the first chunk pair; one-hot work rides on gpsimd.
            with tc.tile_pool(name="em_ps", bufs=2, space="PSUM") as epsp, \
                 tc.tile_pool(name="pa_ps", bufs=1, space="PSUM") as apsp, \
                 tc.tile_pool(name="pg_ps", bufs=2, space="PSUM") as gpsp, \
                 tc.tile_pool(name="fin_ps", bufs=1, space="PSUM") as fpsp:
                def emit_chunk(k):
                    cs = k * 512
                    emps = epsp.tile([T, 512], DT.float32, tag="emps", name="emps")
                    nc.tensor.matmul(emps[:], wot["f"][:], Hf[:, cs:cs + 512],
                                     start=True, stop=False, skip_group_check=True)
                    nc.tensor.matmul(emps[:], wot["b"][:], Hb[:, cs:cs + 512],
                                     start=False, stop=True, skip_group_check=True)
                    Sc.activation(EM[:, cs:cs + 512], emps[:], AF.Exp,
                                  bias=exbias[:])
                    # A-part numerator gather via host-prepped tag one-hot
                    # (end_trans rides the histogram's ls column instead)
                    Vv.affine_mul_reduce(
                        out=junkA[:], accum_out=accA[:, k:k + 1],
                        in0=emps[:], in1=ohm_sb[:, cs:cs + 512],
                        scale=1.0, bias=0.0)

                def ls_slice(t):
                    tok = t * BLOC
                    return lsrow[0:1, tok:tok + BLOC]

                cur = 0
                # G init at t = L-1
                gps_prev = gpsp.tile([T, BLOC], DT.float32, tag="pg", name="pg")
                nc.tensor.matmul(gps_prev[:], eendrow[:], ls_slice(L - 1),
                                 start=True, stop=True, skip_group_check=True)

                at = 1        # next alpha step
                gt = L - 2    # next G step
                for rnd in range(NCH // 2):
                    emit_chunk(rnd)
                    emit_chunk(NCH - 1 - rnd)
                    if rnd == 0:
                        Vv.tensor_scalar(ea[0][:], EM[:, 0:BLOC], estart[:],
                                         None, ALU.mult)
                    while at <= 16 * rnd + 15:
                        pa = apsp.tile([T, BLOC], DT.float32, tag="pa", name="pa")
                        nc.tensor.matmul(pa[:], et_sb[:], ea[cur][:], start=True,
                                         stop=True, skip_group_check=True)
                        cur ^= 1
                        Vv.tensor_tensor(out=ea[cur][:], in0=pa[:],
                                         in1=EM[:, at * BLOC:(at + 1) * BLOC],
                                         op=ALU.mult)
                        at += 1
                    while gt >= max(16 * (NCH - 1 - rnd) - 1, half - 1):
                        # eend injection issued first (no chain dependency) so
                        # it runs off the critical path into the fresh psum
                        gps = gpsp.tile([T, BLOC], DT.float32, tag="pg", name="pg")
                        nc.tensor.matmul(gps[:], eendrow[:], ls_slice(gt),
                                         start=True, stop=False, skip_group_check=True)
                        Vv.tensor_tensor(out=emg[:], in0=gps_prev[:],
                                         in1=EM[:, (gt + 1) * BLOC:(gt + 2) * BLOC],
                                         op=ALU.mult)
                        nc.tensor.matmul(gps[:], ett_sb[:], emg[:], start=False,
                                         stop=True, skip_group_check=True)
                        gps_prev = gps
                        gt -= 1

                # combine
                Vv.tensor_tensor(out=dott[:], in0=gps_prev[:], in1=ea[cur][:],
                                 op=ALU.mult)
                fint = fpsp.tile([1, 64], DT.float32, name="fint")
                nc.tensor.matmul(fint[:, 0:BLOC], ones48col[:], dott[:], start=True,
                                 stop=True, skip_group_check=True)
                Sc.activation(logrow[:], fint[:, 0:BLOC], AF.Ln)
                Vv.tensor_reduce(dsum[:], logrow[:], mybir.AxisListType.X, ALU.add)

                # masksum
                Vv.tensor_reduce(msum[:], m_pcol[:], mybir.AxisListType.X, ALU.add)
                nc.tensor.matmul(fint[:, 32:33], msum[:], ones128col[:], start=True,
                                 stop=True, skip_group_check=True)
                # numerator total: A (start) + BC (accumulate) in one cell
                Vv.tensor_reduce(accA_red[:], accA[:], mybir.AxisListType.X, ALU.add)
                nc.tensor.matmul(fint[:, 34:35], accA_red[:], ones48col[:], start=True,
                                 stop=False, skip_group_check=True)
                nc.tensor.matmul(fint[:, 34:35], accBC[:], ones51col[:], start=False,
                                 stop=True, skip_group_check=True)
                # out[0] = numsum ; out[1] = denomsum
                Vv.tensor_copy(out_sb[:, 0:1], fint[:, 34:35])
                Vv.tensor_scalar(tmp11[:], fint[:, 32:33], c0, None, ALU.mult)
                Vv.tensor_tensor(out=out_sb[:, 1:2], in0=tmp11[:], in1=dsum[:],
                                 op=ALU.add)
                Vv.tensor_copy(out_sb[:, 4:5], fint[:, 32:33])
                Vv.tensor_copy(out_sb[:, 5:6], dsum[:])
            if debug:
                daccA = pp.tile([T, 16], DT.float32, tag="daccA", name="daccA")
                Vv.memset(daccA[:], 0.0)
                Vv.tensor_copy(daccA[:, 0:NCH if NCH <= 16 else 16],
                               accA[:, 0:NCH if NCH <= 16 else 16])
                S.dma_start(out=g_dbg3[:], in_=daccA[:])
            empool.__exit__(None, None, None)
            S.dma_start(out=g_out[:], in_=out_sb[:])

    return nc


# --------------------------------------------------------------------------
# self-contained entry point: kernel(**inputs) -> scalar loss (numpy)
# --------------------------------------------------------------------------

_CACHED = {}


def _get_nc():
    if "nc" not in _CACHED:
        nc = build(L=512, BLOC=32, W=8, V=32000)
        if not nc.is_finalized():
            nc.finalize()
        _CACHED["nc"] = nc
    return _CACHED["nc"]


def kernel(**inputs):
    from concourse.bass_utils import run_bass_kernel_spmd

    B = 256
    BLOC = B // 8
    p = prep_params(inputs)
    in_maps = []
    words = np.asarray(inputs["words"])
    tags = np.asarray(inputs["tags"])
    mask = np.asarray(inputs["mask"])
    for core in range(8):
        sl = slice(core * BLOC, (core + 1) * BLOC)
        d = prep_shard(words[sl], tags[sl], mask[sl])
        d.update(p)
        in_maps.append(d)
    nc = _get_nc()
    res = run_bass_kernel_spmd(nc, in_maps, list(range(8)))
    tot_num = sum(float(res.results[i]["out"][0, 0]) for i in range(8))
    tot_den = sum(float(res.results[i]["out"][0, 1]) for i in range(8))
    loss = (tot_den - tot_num) / B
    return np.float32(loss)



# revision 57
# speedup vs baseline: 1.0052x; 1.0052x over previous
"""BiLSTM-CRF loss kernel for trn2, one core = 32 sequences (data parallel).

Algorithm (validated in proto.py / proto_bf16.py):
- embedding gather via dma_gather(transpose) -> x (128=E, ntok) bf16, t-major
- BiLSTM: all-sigmoid gates (tanh(x)=2*sigma(2x)-1 folded into g-gate weights),
  gates psum accumulate: xproj window matmuls + rank-1 bias + per-step Whh mm
- emissions chunked: psum = WoutT halves @ H, EM = exp(psum + bout - log T)
- numerator: A' = sum OHM*(emis + end x laststep)  [TTR per chunk],
  B+C via (49,48) pair histogram (one-hot matmuls) . trans_ext
- CRF denominator in exp space: alpha chain (t=0..L/2-1) and backward G chain
  (t=L-1..L/2-1) with EEND x laststep injection; denom = sum log(dot) +
  log(T) * masksum
Output per core: (1, 8) f32: [0]=numerator partial sum, [1]=denominator partial.
loss = (sum_den - sum_num) / B   (host combines the 8 cores)
"""
import numpy as np
import ml_dtypes

import concourse.bacc as bacc
import concourse.mybir as mybir
from concourse.tile import TileContext

BF16 = ml_dtypes.bfloat16
F32 = np.float32
AF = mybir.ActivationFunctionType
ALU = mybir.AluOpType
DT = mybir.dt

T = 48

# degree-5 odd minimax fit of tanh on [-1.05, 1.05] (|c| stays below ~0.95;
# max poly err 5.1e-4, under the bf16 rounding already present on H)
TANH_A0 = 0.9964321688911302
TANH_A1 = -0.30413271171334516
TANH_A2 = 0.06905329873698512


def _register_dve_ops():
    """Self-register two fused DVE ops (per-NEFF table, no firmware change):
    TANH_MUL_ANT: out = tanhpoly(in0) * in1   (kills TANH ACT + mult TT)
    AFFINE_MUL_ANT: out = (in0*s0 + s1) * in1 (affine_mul_reduce minus the
    accum drain op)."""
    import dataclasses
    import concourse.dve_ops as D
    from concourse.dve_spec import Spec, Src0, Src1, C0, C1, C2, sq, lower
    from concourse.dve_ops import has_src1
    from concourse.dve_uop import DveOpSpec

    def _register(name, spec, subdim=False):
        for o in D.OPS:
            if o.name == name:
                return o
        op = D.DveOp(name, spec, subdim=subdim, uops_sha={})
        D.OPS.append(op)
        D.CUSTOM_DVE_SPECS[name] = spec
        D._SUB_OPCODE_FOR_NAME[name] = D._CUSTOM_DVE_ROW_BASE + len(D.OPS) - 1
        shas = {}
        for ver in ("v3", "v4"):
            s = DveOpSpec(name=name, opcode=D.get_dve_sub_opcode(name),
                          uops=lower(spec, ver=ver), rd1_en=has_src1(spec))
            shas[ver] = s.sha(ver)
        op2 = dataclasses.replace(op, uops_sha=shas)
        D.OPS[-1] = op2
        D.CUSTOM_DVE_SPECS[name] = op2.spec
        return op2

    u = sq(Src0)
    tanh_mul = _register(
        "TANH_MUL_ANT",
        Spec(
            body=Src0 * (C0 + u * (C1 + u * C2)) * Src1,
            reference=lambda in0, in1, s0, s1, imm2: (
                in0.astype(np.float32)
                * (s0 + in0.astype(np.float32) ** 2
                   * (s1 + in0.astype(np.float32) ** 2 * imm2))
                * in1.astype(np.float32)),
        ))
    aff_mul = _register(
        "AFFINE_MUL_ANT",
        Spec(
            body=(Src0 * C0 + C1) * Src1,
            reference=lambda in0, in1, s0, s1, imm2: (
                (in0.astype(np.float32) * s0 + s1) * in1.astype(np.float32)),
        ))
    return tanh_mul, aff_mul


# --------------------------------------------------------------------------
# host-side preparation
# --------------------------------------------------------------------------

def prep_params(inp):
    """Build replicated parameter arrays (numpy) from raw inputs."""
    p = {}
    p["emb"] = np.ascontiguousarray(inp["emb"]).astype(BF16)

    def mk(Wih, Whh, bih, bhh):
        def reorder(W):
            i, f, g, o = np.split(np.asarray(W, F32), 4, 0)
            return np.concatenate([i, f, o, 2.0 * g], 0)
        WihT = np.ascontiguousarray(reorder(Wih).T).astype(BF16)   # (128, 512)
        WhhT = np.ascontiguousarray(reorder(Whh).T).astype(BF16)   # (128, 512)
        b = np.asarray(bih, F32) + np.asarray(bhh, F32)
        bi, bf_, bg, bo = np.split(b, 4)
        bias = np.concatenate([bi, bf_, bo, 2.0 * bg]).reshape(1, -1).astype(BF16)
        return WihT, WhhT, bias

    p["wiht_f"], p["whht_f"], p["bias_f"] = mk(inp["Wih_f"], inp["Whh_f"], inp["bih_f"], inp["bhh_f"])
    p["wiht_b"], p["whht_b"], p["bias_b"] = mk(inp["Wih_b"], inp["Whh_b"], inp["bih_b"], inp["bhh_b"])
    Wout = np.asarray(inp["Wout"], F32)     # (48, 256)
    H = Wout.shape[1] // 2
    p["wot_f"] = np.ascontiguousarray(Wout[:, :H].T).astype(BF16)   # (128, 48)
    p["wot_b"] = np.ascontiguousarray(Wout[:, H:].T).astype(BF16)
    c0 = np.log(T)
    p["exbias"] = (np.asarray(inp["bout"], F32) - c0).reshape(T, 1).astype(F32)
    trans = np.asarray(inp["trans"], F32)
    p["et"] = np.exp(trans).astype(BF16)                     # (48,48) lhsT alpha
    p["ett"] = np.ascontiguousarray(np.exp(trans).T).astype(BF16)  # lhsT G
    p["estart"] = np.exp(np.asarray(inp["start_trans"], F32)).reshape(T, 1).astype(F32)
    p["eendrow"] = np.exp(np.asarray(inp["end_trans"], F32)).reshape(1, T).astype(BF16)
    # bout is absent from the emission psum that the A-gather reads (it only
    # enters via the exp bias), so fold it in via the pair-histogram: every
    # masked position contributes exactly one CNT count with its cur-tag.
    # Row 50 counts last-step positions (ls indicator in ohp col 50), which
    # carries end_trans into the numerator without per-chunk endrow matmuls.
    p["transext"] = np.concatenate([
        trans + np.asarray(inp["bout"], F32)[None, :],
        np.asarray(inp["start_trans"], F32)[None, :]
        + np.asarray(inp["bout"], F32)[None, :],
        np.zeros((1, T), F32),
        np.asarray(inp["end_trans"], F32)[None, :],
    ], 0).astype(F32)  # (51, 48): trans+bout, start+bout, zero, end
    # gate-region indicator for the rank-4 bias matmul: (4, 1024) over the
    # psum window [4 gates x REG=256 f32 cols]; ind[g, c] = (c // 256 == g)
    reg = 256
    p["ind"] = (np.arange(4 * reg)[None, :] // reg ==
                np.arange(4)[:, None]).astype(BF16)
    return p


def prep_shard(words, tags, mask):
    """Per-core input arrays. words/tags/mask: (b, L)."""
    b, L = words.shape
    ntok = b * L
    npch = ntok // 128
    w_tm = np.ascontiguousarray(words.T).reshape(-1)
    tags_tm = np.ascontiguousarray(tags.T).reshape(-1)
    m_tm = np.ascontiguousarray(mask.T).reshape(-1).astype(F32)

    d = {}
    gi = w_tm.astype(np.int16).reshape(ntok // 16, 16).T          # (16, ntok/16)
    d["gidx"] = np.ascontiguousarray(np.tile(gi, (8, 1))).astype(np.int16)
    tm_masked = np.where(m_tm > 0, tags_tm, 99).astype(F32)
    tprev = np.concatenate([np.full(b, 48, F32), tags_tm[:-b].astype(F32)])
    d["mask_pc"] = np.ascontiguousarray(m_tm.reshape(-1, 128).T).astype(BF16)
    m_pad = np.pad(m_tm, (0, b))
    ls = (m_tm - m_pad[b:]).astype(F32)
    d["lsrow"] = ls.astype(BF16).reshape(1, ntok)
    # host-prepped one-hots (pure functions of tags/mask):
    # ohm: (48, ntok) onehot of masked cur-tag, token-chunk layout (A-gather)
    d["ohm"] = (np.arange(T, dtype=F32)[:, None] ==
                tm_masked[None, :]).astype(BF16)
    # ohp: (128, 64*npch) pcol onehot of prev-tag (48 = seq start) with the
    # last-step indicator in col 50; ohc: (128, 48*npch) masked cur-tag
    ohp = (tprev[:, None] == np.arange(64, dtype=F32)[None, :]).astype(F32)
    ohp[:, 50] = ls
    d["ohp"] = np.ascontiguousarray(
        ohp.reshape(npch, 128, 64).transpose(1, 0, 2).reshape(128, npch * 64)
    ).astype(BF16)
    ohc = (tm_masked[:, None] == np.arange(T, dtype=F32)[None, :]).astype(F32)
    d["ohc"] = np.ascontiguousarray(
        ohc.reshape(npch, 128, T).transpose(1, 0, 2).reshape(128, npch * T)
    ).astype(BF16)
    return d


# --------------------------------------------------------------------------
# device kernel builder
# --------------------------------------------------------------------------

def build(L=512, BLOC=32, W=8, V=32000, debug=False, phases=("lstm", "hist", "emis", "crf")):
    ntok = L * BLOC
    NW = L // W
    half = L // 2
    NCH = ntok // 512          # emission chunks
    NPCH = ntok // 128         # one-hot pchunks
    c0 = float(np.log(T))

    tanh_mul, aff_mul = _register_dve_ops()
    nc = bacc.Bacc()
    dp = nc.declare_dram_parameter
    g_gidx = dp("gidx", [128, ntok // 16], DT.int16, isOutput=False)
    g_ohm = dp("ohm", [T, ntok], DT.bfloat16, isOutput=False)
    g_ohp = dp("ohp", [128, 64 * NPCH], DT.bfloat16, isOutput=False)
    g_ohc = dp("ohc", [128, T * NPCH], DT.bfloat16, isOutput=False)
    g_maskpc = dp("mask_pc", [128, ntok // 128], DT.bfloat16, isOutput=False)
    g_lsrow = dp("lsrow", [1, ntok], DT.bfloat16, isOutput=False)
    g_emb = dp("emb", [V, 128], DT.bfloat16, isOutput=False)
    g_w = {}
    for nm in ("wiht_f", "whht_f", "wiht_b", "whht_b"):
        g_w[nm] = dp(nm, [128, 512], DT.bfloat16, isOutput=False)
    g_bias = {d: dp(f"bias_{d}", [1, 512], DT.bfloat16, isOutput=False) for d in "fb"}
    g_wot = {d: dp(f"wot_{d}", [128, T], DT.bfloat16, isOutput=False) for d in "fb"}
    g_exbias = dp("exbias", [T, 1], DT.float32, isOutput=False)
    g_et = dp("et", [T, T], DT.bfloat16, isOutput=False)
    g_ett = dp("ett", [T, T], DT.bfloat16, isOutput=False)
    g_estart = dp("estart", [T, 1], DT.float32, isOutput=False)
    g_eendrow = dp("eendrow", [1, T], DT.bfloat16, isOutput=False)
    g_transext = dp("transext", [51, T], DT.float32, isOutput=False)
    g_out = dp("out", [1, 8], DT.float32, isOutput=True)
    if debug:
        g_dbg1 = dp("dbg1", [T, 512], DT.float32, isOutput=True)
        g_dbg2 = dp("dbg2", [T, 512], DT.float32, isOutput=True)
        g_dbg3 = dp("dbg3", [T, 16], DT.float32, isOutput=True)

    with TileContext(nc) as tc:
        with tc.tile_pool(name="persist", bufs=1) as pp:
            # ---- persistent SBUF tiles
            Hf = pp.tile([128, ntok], DT.bfloat16, tag="Hf", name="Hf")
            Hb = pp.tile([128, ntok], DT.bfloat16, tag="Hb", name="Hb")
            wiht = {}
            whht = {}
            bias = {}
            wot = {}
            for d in "fb":
                wiht[d] = pp.tile([128, 512], DT.bfloat16, tag=f"wiht{d}", name=f"wiht{d}")
                whht[d] = pp.tile([128, 512], DT.bfloat16, tag=f"whht{d}", name=f"whht{d}")
                bias[d] = pp.tile([1, 512], DT.bfloat16, tag=f"bias{d}", name=f"bias{d}")
                wot[d] = pp.tile([128, T], DT.bfloat16, tag=f"wot{d}", name=f"wot{d}")
            exbias = pp.tile([T, 1], DT.float32, tag="exbias", name="exbias")
            et_sb = pp.tile([T, T], DT.bfloat16, tag="et", name="et")
            ett_sb = pp.tile([T, T], DT.bfloat16, tag="ett", name="ett")
            estart = pp.tile([T, 1], DT.float32, tag="estart", name="estart")
            eendrow = pp.tile([1, T], DT.bfloat16, tag="eendrow", name="eendrow")
            transext = pp.tile([51, T], DT.float32, tag="transext", name="transext")
            ohm_sb = pp.tile([T, ntok], DT.bfloat16, tag="ohm", name="ohm")
            ohp_sb = pp.tile([128, 64 * NPCH], DT.bfloat16, tag="ohp", name="ohp")
            ohc_sb = pp.tile([128, T * NPCH], DT.bfloat16, tag="ohc", name="ohc")
            m_pcol = pp.tile([128, NPCH], DT.bfloat16, tag="mpcol", name="mpcol")
            lsrow = pp.tile([1, ntok], DT.bfloat16, tag="lsrow", name="lsrow")
            # small constants
            ones48row = pp.tile([1, T], DT.float32, tag="ones48row", name="ones48row")
            ones128row = pp.tile([1, 128], DT.float32, tag="ones128row", name="ones128row")
            onesrow512 = pp.tile([1, 512], DT.float32, tag="onesrow512", name="onesrow512")
            onesrow512b = pp.tile([1, 512], DT.bfloat16, tag="onesrow512b", name="onesrow512b")
            ones48col = pp.tile([T, 1], DT.float32, tag="ones48col", name="ones48col")
            ones51col = pp.tile([51, 1], DT.float32, tag="ones51col", name="ones51col")
            ones128col = pp.tile([128, 1], DT.float32, tag="ones128col", name="ones128col")
            # LSTM state
            cst = {d: pp.tile([128, BLOC], DT.float32, tag=f"c{d}", name=f"c{d}") for d in "fb"}
            tmp1 = {d: pp.tile([128, BLOC], DT.float32, tag=f"tmp1{d}", name=f"tmp1{d}") for d in "fb"}
            tmp2 = {d: pp.tile([128, BLOC], DT.bfloat16, tag=f"tmp2{d}", name=f"tmp2{d}") for d in "fb"}
            tct = {d: pp.tile([128, BLOC], DT.bfloat16, tag=f"tct{d}", name=f"tct{d}") for d in "fb"}
            jacc = {d: pp.tile([128, 1], DT.float32, tag=f"jacc{d}", name=f"jacc{d}") for d in "fb"}
            # numerator accumulators
            accA = pp.tile([T, NCH], DT.float32, tag="accA", name="accA")
            accA_red = pp.tile([T, 1], DT.float32, tag="accAred", name="accAred")
            accBC = pp.tile([51, 1], DT.float32, tag="accBC", name="accBC")
            junkA = pp.tile([T, 512], DT.bfloat16, tag="junkA", name="junkA")
            junkBC = pp.tile([51, T], DT.float32, tag="junkBC", name="junkBC")
            msum = pp.tile([128, 1], DT.float32, tag="msum", name="msum")
            # CRF tiles
            ea = [pp.tile([T, BLOC], DT.bfloat16, tag=f"ea{i}", name=f"ea{i}") for i in range(2)]
            emg = pp.tile([T, BLOC], DT.bfloat16, tag="emg", name="emg")
            dott = pp.tile([T, BLOC], DT.float32, tag="dott", name="dott")
            logrow = pp.tile([1, BLOC], DT.float32, tag="logrow", name="logrow")
            dsum = pp.tile([1, 1], DT.float32, tag="dsum", name="dsum")
            tmp11 = pp.tile([1, 1], DT.float32, tag="tmp11", name="tmp11")
            out_sb = pp.tile([1, 8], DT.float32, tag="outsb", name="outsb")

            # ---- input DMAs
            S = nc.sync
            for d in "fb":
                S.dma_start(out=wiht[d][:], in_=g_w[f"wiht_{d}"][:])
                S.dma_start(out=whht[d][:], in_=g_w[f"whht_{d}"][:])
                S.dma_start(out=bias[d][:], in_=g_bias[d][:])
                S.dma_start(out=wot[d][:], in_=g_wot[d][:])
            S.dma_start(out=exbias[:], in_=g_exbias[:])
            S.dma_start(out=et_sb[:], in_=g_et[:])
            S.dma_start(out=ett_sb[:], in_=g_ett[:])
            S.dma_start(out=estart[:], in_=g_estart[:])
            S.dma_start(out=eendrow[:], in_=g_eendrow[:])
            S.dma_start(out=transext[:], in_=g_transext[:])
            S.dma_start(out=m_pcol[:], in_=g_maskpc[:])
            S.dma_start(out=lsrow[:], in_=g_lsrow[:])

            # constants
            Vv = nc.vector
            Sc = nc.scalar
            Vv.memset(ones48row[:], 1.0)
            Vv.memset(ones128row[:], 1.0)
            Vv.memset(onesrow512[:], 1.0)
            Vv.memset(onesrow512b[:], 1.0)
            Vv.memset(ones48col[:], 1.0)
            Vv.memset(ones51col[:], 1.0)
            Vv.memset(ones128col[:], 1.0)
            Vv.memset(accA[:], 0.0)
            Vv.memset(out_sb[:], 0.0)
            for d in "fb":
                Vv.memset(cst[d][:], 0.0)

            # ---------------- LSTM ----------------
            emis_lvl = 4
            for ph in phases:
                if ph.startswith("emis") and len(ph) > 4:
                    emis_lvl = int(ph[4:])
            do_lstm = "lstm" in phases
            do_hist = "hist" in phases
            do_emis = "emis" in phases
            do_crf = "crf" in phases
            do_emis = do_emis or any(ph.startswith("emis") for ph in phases)
            if not do_lstm:
                Vv.memset(Hf[:], 0.0)
                Vv.memset(Hb[:], 0.0)
            REG = 32 * W      # region width per gate
            Hdir = {"f": Hf, "b": Hb}
            # issue the embedding gathers first (gpsimd queue), then the bulky
            # one-hot DMAs so they don't delay the gather start
            xpool_cm = tc.tile_pool(name="xpool", bufs=1)
            xp = xpool_cm.__enter__()
            x = xp.tile([128, ntok], DT.bfloat16, tag="x", name="x")
            gidx = xp.tile([128, ntok // 16], DT.int16, tag="gidx", name="gidx")
            S.dma_start(out=gidx[:], in_=g_gidx[:])
            GCH = min(ntok, 1024)
            _ng = ntok // GCH
            _order = []
            for _i in range((_ng + 1) // 2):
                _order.append(_i)
                if _ng - 1 - _i != _i:
                    _order.append(_ng - 1 - _i)
            for gc in _order:
                nc.gpsimd.dma_gather(
                    out_ap=x[:, gc * GCH:(gc + 1) * GCH].rearrange(
                        "p (o n) -> p o n", o=1),
                    in_ap=g_emb[:],
                    idxs_ap=gidx[:, gc * (GCH // 16):(gc + 1) * (GCH // 16)],
                    num_idxs=GCH,
                    num_idxs_reg=GCH,
                    elem_size=128,
                    transpose=True,
                    single_packet=False,
                )
            S.dma_start(out=ohp_sb[:], in_=g_ohp[:])
            S.dma_start(out=ohc_sb[:], in_=g_ohc[:])
            S.dma_start(out=ohm_sb[:], in_=g_ohm[:])
            # pair histogram during the gather window (tensor is mostly idle
            # here; the psum bank is freed before the LSTM pool opens)
            with tc.tile_pool(name="cnt_ps", bufs=1, space="PSUM") as cpsp:
                cntps = cpsp.tile([64, T], DT.float32, name="cntps")
                for q in range(NPCH):
                    nc.tensor.matmul(cntps[:], ohp_sb[:, 64 * q:64 * (q + 1)],
                                     ohc_sb[:, T * q:T * (q + 1)],
                                     start=(q == 0), stop=(q == NPCH - 1),
                                     skip_group_check=True)
                Vv.affine_mul_reduce(
                    out=junkBC[:], accum_out=accBC[:],
                    in0=transext[:], in1=cntps[0:51, :],
                    scale=1.0, bias=0.0)
            with tc.tile_pool(name="lstm_ps", bufs=2, space="PSUM") as lpsp, \
                 tc.tile_pool(name="lstm_sb", bufs=3) as lsb:
                def alloc_boundary(w):
                    """Allocate window-w gate psums; return per-gate groups of
                    deferred xproj/bias matmul closures. Each group is gated on
                    a junk 1-col copy from the current H column (WAW hazard on
                    the gate region, erased by the group's start=True matmul)
                    so the greedy scheduler cannot run them all at window
                    start and starve the recurrence of tensor slots."""
                    pfd = {}
                    x0d = {}
                    for d in "fb":
                        pfd[d] = lpsp.tile([128, 4 * REG], DT.float32,
                                           tag=f"pf{d}", name=f"pf{d}")
                        x0d[d] = (w * W * BLOC) if d == "f" \
                            else (L - (w + 1) * W) * BLOC
                    groups = []
                    for gi in range(4):
                        for d in "fb":
                            cls = []

                            def xmm(d=d, gi=gi, pfd=pfd, x0d=x0d):
                                nc.tensor.matmul(
                                    pfd[d][:, gi * REG:(gi + 1) * REG],
                                    wiht[d][:, gi * 128:(gi + 1) * 128],
                                    x[:, x0d[d]:x0d[d] + W * BLOC],
                                    start=((gi * REG * 4) % 2048 == 0),
                                    stop=False, skip_group_check=True)

                            def bmm(d=d, gi=gi, pfd=pfd):
                                nc.tensor.matmul(
                                    pfd[d][:, gi * REG:(gi + 1) * REG],
                                    bias[d][0:1, gi * 128:(gi + 1) * 128],
                                    onesrow512b[0:1, 0:W * BLOC],
                                    start=False, stop=False, skip_group_check=True)
                            cls.append(xmm)
                            cls.append(bmm)
                            groups.append((d, gi, cls))
                    return pfd, groups

                pf = None
                if do_lstm:
                    pf, groups0 = alloc_boundary(0)
                    for _, _, cls in groups0:
                        for c in cls:
                            c()
                for w in range(NW if do_lstm else 0):
                    if w + 1 < NW:
                        pf_next, pending = alloc_boundary(w + 1)
                    else:
                        pf_next, pending = None, []
                    for s in range(W):
                        for d in "fb":
                            if d == "f":
                                t = w * W + s
                                slot = s
                                tprev_col = (t - 1) * BLOC
                                first = (t == 0)
                            else:
                                t = L - 1 - (w * W + s)
                                slot = W - 1 - s
                                tprev_col = (t + 1) * BLOC
                                first = (t == L - 1)
                            Hd = Hdir[d]
                            pfd = pf[d]
                            if not first:
                                for gi in range(4):
                                    nc.tensor.matmul(
                                        pfd[:, gi * REG + slot * 32: gi * REG + (slot + 1) * 32],
                                        whht[d][:, gi * 128:(gi + 1) * 128],
                                        Hd[:, tprev_col:tprev_col + BLOC],
                                        start=False, stop=True, skip_group_check=True)
                            # sigma over the 4 gate slices
                            Sg = lsb.tile([128, 128], DT.bfloat16, tag=f"S{d}", name=f"S{d}")
                            pf3 = pfd[:].rearrange("p (g n) -> p g n", g=4)
                            Sc.activation(
                                Sg[:].rearrange("p (g n) -> p g n", g=4),
                                pf3[:, :, slot * 32:(slot + 1) * 32],
                                AF.Sigmoid)
                            # c update
                            if first:
                                Vv._custom_dve(
                                    aff_mul, out=cst[d][:],
                                    in0=Sg[:, 96:128], in1=Sg[:, 0:32],
                                    s0=2.0, s1=-1.0)
                            else:
                                Vv.tensor_tensor(out=tmp1[d][:], in0=Sg[:, 32:64],
                                                 in1=cst[d][:], op=ALU.mult)
                                Vv._custom_dve(
                                    aff_mul, out=tmp2[d][:],
                                    in0=Sg[:, 96:128], in1=Sg[:, 0:32],
                                    s0=2.0, s1=-1.0)
                                Vv.tensor_tensor(out=cst[d][:], in0=tmp1[d][:],
                                                 in1=tmp2[d][:], op=ALU.add)
                            # h = sigma(o) * tanh(c) in one fused DVE op
                            Vv._custom_dve(
                                tanh_mul, out=Hd[:, t * BLOC:(t + 1) * BLOC],
                                in0=cst[d][:], in1=Sg[:, 64:96],
                                s0=TANH_A0, s1=TANH_A1, imm2=TANH_A2)
                        # release one per-dir gate-group of next-window
                        # xproj/bias matmuls, gated on this step's fresh H col
                        # (junk copy on the scalar engine, which has slack)
                        if pending:
                            gd, gi, cls = pending.pop(0)
                            tcur = (w * W + s) if gd == "f" \
                                else (L - 1 - (w * W + s))
                            Vv.tensor_copy(
                                pf_next[gd][:, gi * REG:gi * REG + 1],
                                Hdir[gd][:, tcur * BLOC:tcur * BLOC + 1])
                            for c in cls:
                                c()
                    pf = pf_next

            xpool_cm.__exit__(None, None, None)   # x is dead after the LSTM
            empool = tc.tile_pool(name="empool", bufs=1)
            emp = empool.__enter__()
            EM = emp.tile([T, ntok], DT.bfloat16, tag="EM", name="EM")
            # ------- merged emissions + CRF (interleaved) -------
            # EM chunks are emitted from both ends inward so the alpha chain
            # (consuming t ascending) and the G chain (descending) can start
            # right after the first chunk pair; one-hot work rides on gpsimd.
            with tc.tile_pool(name="em_ps", bufs=2, space="PSUM") as epsp, \
                 tc.tile_pool(name="pa_ps", bufs=1, space="PSUM") as apsp, \
                 tc.tile_pool(name="pg_ps", bufs=2, space="PSUM") as gpsp, \
                 tc.tile_pool(name="fin_ps", bufs=1, space="PSUM") as fpsp:
                def emit_chunk(k):
                    cs = k * 512
                    emps = epsp.tile([T, 512], DT.float32, tag="emps", name="emps")
                    nc.tensor.matmul(emps[:], wot["f"][:], Hf[:, cs:cs + 512],
                                     start=True, stop=False, skip_group_check=True)
                    nc.tensor.matmul(emps[:], wot["b"][:], Hb[:, cs:cs + 512],
                                     start=False, stop=True, skip_group_check=True)
                    Sc.activation(EM[:, cs:cs + 512], emps[:], AF.Exp,
                                  bias=exbias[:])
                    # A-part numerator gather via host-prepped tag one-hot
                    # (end_trans rides the histogram's ls column instead)
                    Vv.affine_mul_reduce(
                        out=junkA[:], accum_out=accA[:, k:k + 1],
                        in0=emps[:], in1=ohm_sb[:, cs:cs + 512],
                        scale=1.0, bias=0.0)

                def ls_slice(t):
                    tok = t * BLOC
                    return lsrow[0:1, tok:tok + BLOC]

                cur = 0
                # G init at t = L-1
                gps_prev = gpsp.tile([T, BLOC], DT.float32, tag="pg", name="pg")
                nc.tensor.matmul(gps_prev[:], eendrow[:], ls_slice(L - 1),
                                 start=True, stop=True, skip_group_check=True)

                at = 1        # next alpha step
                gt = L - 2    # next G step
                for rnd in range(NCH // 2):
                    emit_chunk(rnd)
                    emit_chunk(NCH - 1 - rnd)
                    if rnd == 0:
                        Vv.tensor_scalar(ea[0][:], EM[:, 0:BLOC], estart[:],
                                         None, ALU.mult)
                    while at <= 16 * rnd + 15:
                        pa = apsp.tile([T, BLOC], DT.float32, tag="pa", name="pa")
                        nc.tensor.matmul(pa[:], et_sb[:], ea[cur][:], start=True,
                                         stop=True, skip_group_check=True)
                        cur ^= 1
                        Vv.tensor_tensor(out=ea[cur][:], in0=pa[:],
                                         in1=EM[:, at * BLOC:(at + 1) * BLOC],
                                         op=ALU.mult)
                        at += 1
                    while gt >= max(16 * (NCH - 1 - rnd) - 1, half - 1):
                        # eend injection issued first (no chain dependency) so
                        # it runs off the critical path into the fresh psum
                        gps = gpsp.tile([T, BLOC], DT.float32, tag="pg", name="pg")
                        nc.tensor.matmul(gps[:], eendrow[:], ls_slice(gt),
                                         start=True, stop=False, skip_group_check=True)
                        Vv.tensor_tensor(out=emg[:], in0=gps_prev[:],
                                         in1=EM[:, (gt + 1) * BLOC:(gt + 2) * BLOC],
                                         op=ALU.mult)
                        nc.tensor.matmul(gps[:], ett_sb[:], emg[:], start=False,
                                         stop=True, skip_group_check=True)
                        gps_prev = gps
                        gt -= 1

                # combine
                Vv.tensor_tensor(out=dott[:], in0=gps_prev[:], in1=ea[cur][:],
                                 op=ALU.mult)
                fint = fpsp.tile([1, 64], DT.float32, name="fint")
                nc.tensor.matmul(fint[:, 0:BLOC], ones48col[:], dott[:], start=True,
                                 stop=True, skip_group_check=True)
                Sc.activation(logrow[:], fint[:, 0:BLOC], AF.Ln)
                Vv.tensor_reduce(dsum[:], logrow[:], mybir.AxisListType.X, ALU.add)

                # masksum
                Vv.tensor_reduce(msum[:], m_pcol[:], mybir.AxisListType.X, ALU.add)
                nc.tensor.matmul(fint[:, 32:33], msum[:], ones128col[:], start=True,
                                 stop=True, skip_group_check=True)
                # numerator total: A (start) + BC (accumulate) in one cell
                Vv.tensor_reduce(accA_red[:], accA[:], mybir.AxisListType.X, ALU.add)
                nc.tensor.matmul(fint[:, 34:35], accA_red[:], ones48col[:], start=True,
                                 stop=False, skip_group_check=True)
                nc.tensor.matmul(fint[:, 34:35], accBC[:], ones51col[:], start=False,
                                 stop=True, skip_group_check=True)
                # out[0] = numsum ; out[1] = denomsum
                Vv.tensor_copy(out_sb[:, 0:1], fint[:, 34:35])
                Vv.tensor_scalar(tmp11[:], fint[:, 32:33], c0, None, ALU.mult)
                Vv.tensor_tensor(out=out_sb[:, 1:2], in0=tmp11[:], in1=dsum[:],
                                 op=ALU.add)
                Vv.tensor_copy(out_sb[:, 4:5], fint[:, 32:33])
                Vv.tensor_copy(out_sb[:, 5:6], dsum[:])
            if debug:
                daccA = pp.tile([T, 16], DT.float32, tag="daccA", name="daccA")
                Vv.memset(daccA[:], 0.0)
                Vv.tensor_copy(daccA[:, 0:NCH if NCH <= 16 else 16],
                               accA[:, 0:NCH if NCH <= 16 else 16])
                S.dma_start(out=g_dbg3[:], in_=daccA[:])
            empool.__exit__(None, None, None)
            S.dma_start(out=g_out[:], in_=out_sb[:])

    return nc


# --------------------------------------------------------------------------
# self-contained entry point: kernel(**inputs) -> scalar loss (numpy)
# --------------------------------------------------------------------------

_CACHED = {}


def _get_nc():
    if "nc" not in _CACHED:
        nc = build(L=512, BLOC=32, W=8, V=32000)
        if not nc.is_finalized():
            nc.finalize()
        _CACHED["nc"] = nc
    return _CACHED["nc"]


def kernel(**inputs):
    from concourse.bass_utils import run_bass_kernel_spmd

    B = 256
    BLOC = B // 8
    p = prep_params(inputs)
    in_maps = []
    words = np.asarray(inputs["words"])
    tags = np.asarray(inputs["tags"])
    mask = np.asarray(inputs["mask"])
    for core in range(8):
        sl = slice(core * BLOC, (core + 1) * BLOC)
        d = prep_shard(words[sl], tags[sl], mask[sl])
        d.update(p)
        in_maps.append(d)
    nc = _get_nc()
    res = run_bass_kernel_spmd(nc, in_maps, list(range(8)))
    tot_num = sum(float(res.results[i]["out"][0, 0]) for i in range(8))
    tot_den = sum(float(res.results[i]["out"][0, 1]) for i in range(8))
    loss = (tot_den - tot_num) / B
    return np.float32(loss)

